# revision 54
# baseline (speedup 1.0000x reference)
"""MultiHeadClassifier (MoE routing) Trainium2 kernel.

Problem: B=65536 samples of dim D=1024, each routed by task_id to one of
T=16 two-layer heads (D->H=128 relu -> C=10). The dense reference computes
all 16 heads for every sample (275 GFLOP); here we route on the host and
compute only each sample's own head (~17 GFLOP), data-parallel with 2 tasks
per NeuronCore across 8 cores.

Strategy:
  - Host: stable-argsort samples by task; each core c owns tasks (2c, 2c+1).
    Every task segment is padded to a common M_task rows (multiple of 128) so
    the single SPMD program is identical across cores. Pad rows point at row 0
    (results discarded on unshard).
  - Host pre-transposes each core's gathered rows to xT [2, D, M_task] so the
    contraction dim D lies on SBUF partitions with contiguous DMA lines.
  - Device per (slot, m-tile of 512): 8 accumulating matmuls (W1 d-chunk
    [128,128] stationary, xT chunk [128,512] moving) -> PSUM [H=128, 512];
    ScalarE fused bias+ReLU into SBUF; one matmul with W2 [128,10] as lhsT ->
    PSUM [10,512]; ScalarE fused bias; DMA out [10, m] tiles.
  - Host scatters per-task outputs back to the original order.

MM_DTYPE selects matmul operand precision (measured on trn2, 8 cores):
  - "bf16" (default): host casts x/W to bf16; halves DMA bytes and doubles
    PE stream rate. ~70 us, rel err ~3.4e-3 (f32 accumulation in PSUM).
  - "f32r": TF32-mode matmul (fp32 bits, PE rounds mantissa internally).
    ~107 us, rel err ~2.1e-4.
  - "f32": exact fp32 two-pass matmul. ~150 us, rel err ~1.2e-7.
"""

import sys

import numpy as np

for _p in ("/opt/trn_rl_repo", "/root/.axon_site/_ro/trn_rl_repo"):
    if _p not in sys.path:
        sys.path.append(_p)

import concourse.bacc as bacc
import concourse.mybir as mybir
from concourse.bass_utils import run_bass_kernel_spmd
from concourse.tile import TileContext

B, D, T, H, C = 65536, 1024, 16, 128, 10
N_CORES = 8
S = T // N_CORES  # task slots per core = 2
DC = D // 128  # d-chunks of 128 = 8
MT = 512  # m-tile (max fp32 moving free dim)
X_BUFS = 2

MM_DTYPE = "bf16"

_F32 = mybir.dt.float32


def _mm_dt(mm_dtype):
    return {
        "f32": mybir.dt.float32,
        "f32r": mybir.dt.float32r,
        "bf16": mybir.dt.bfloat16,
    }[mm_dtype]


def _np_in_dt(mm_dtype):
    import ml_dtypes

    return np.dtype(ml_dtypes.bfloat16) if mm_dtype == "bf16" else np.dtype(np.float32)


def _chunks(total, step, merge_tail=0):
    """Split [0, total) into (start, len) chunks of `step` plus remainder.

    A final chunk smaller than merge_tail is merged into the previous one.
    """
    out = []
    p = 0
    while p < total:
        c = min(step, total - p)
        out.append((p, c))
        p += c
    if merge_tail and len(out) > 1 and out[-1][1] < merge_tail:
        p0, c0 = out[-2]
        p1, c1 = out[-1]
        out[-2:] = [(p0, c0 + c1)]
    return out


def _blocks(total, step):
    """Like _chunks but ends with a small (<=512) final block so the
    compute tail after the last DMA is short."""
    out = []
    p = 0
    rem = total
    while rem > 0:
        if rem <= 512 or rem <= step:
            c = rem
        elif rem <= step + 512:
            c = rem - 512
        else:
            c = step
        out.append((p, c))
        p += c
        rem -= c
    return out


def _build(M_task, mm_dtype=MM_DTYPE):
    dt_in = _mm_dt(mm_dtype)
    # x DMA block (samples)
    XB = 2048
    x_bufs = 3 if mm_dtype == "bf16" else X_BUFS
    # bf16 matmuls double throughput when the HAM clock-gate is open; keep
    # the PE duty high with warmup + filler matmuls. f32r matmuls are
    # SBUF-stream-bound (same speed warm or cold) -> fillers only hurt.
    n_warmup = 16 if mm_dtype == "bf16" else 0
    n_fill = 0
    nc = bacc.Bacc(None, target_bir_lowering=False)
    xT = nc.declare_dram_parameter("xT", [S, D, M_task], dt_in, isOutput=False)
    # w1 arrives host-repacked as [S, 128, DC*H]: partition-major, 4KB/row
    w1 = nc.declare_dram_parameter("w1", [S, 128, DC * H], dt_in, isOutput=False)
    b1 = nc.declare_dram_parameter("b1", [S, H], _F32, isOutput=False)
    w2 = nc.declare_dram_parameter("w2", [S, H, C], dt_in, isOutput=False)
    b2 = nc.declare_dram_parameter("b2", [S, C], _F32, isOutput=False)
    outT = nc.declare_dram_parameter("outT", [S, C, M_task], _F32, isOutput=True)

    relu = mybir.ActivationFunctionType.Relu

    with TileContext(nc) as tc:
        with (
            tc.tile_pool(name="wpool", bufs=2) as wpool,
            tc.tile_pool(name="xpool", bufs=x_bufs) as xpool,
            tc.tile_pool(name="hpool", bufs=6) as hpool,
            tc.tile_pool(name="opool", bufs=2) as opool,
            tc.tile_pool(name="warm", bufs=1) as warm,
            tc.tile_pool(name="psum1", bufs=5, space="PSUM") as psum1,
            tc.tile_pool(name="psum2", bufs=2, space="PSUM") as psum2,
            tc.tile_pool(name="psumw", bufs=1, space="PSUM") as psumw,
        ):  # PSUM banks: 5 + 2 + 1 = 8
            # PE warmup: dummy matmuls release the HAM clock-gate (~3.4us of
            # sustained PE busy) while the first x block streams in.
            if n_warmup or n_fill:
                wsrc = warm.tile([128, MT], _F32, tag="wsrc")
                nc.gpsimd.memset(wsrc[:], 0.0)
                wv = wsrc[:].bitcast(dt_in)
                wps = psumw.tile([128, MT], _F32, tag="wps")

            def fill_mm(n):
                for _ in range(n):
                    nc.tensor.matmul(
                        wps[:], wv[:, :128], wv[:, :MT], start=True, stop=True
                    )

            fill_mm(n_warmup)
            for s in range(S):
                w1t = wpool.tile([128, DC, H], dt_in, tag="w1")
                nc.sync.dma_start(
                    w1t, w1[s].rearrange("p (dc h) -> p dc h", dc=DC)
                )
                b1t = wpool.tile([H, 1], _F32, tag="b1")
                nc.sync.dma_start(b1t, b1[s][:, None])
                w2t = wpool.tile([H, C], dt_in, tag="w2")
                nc.sync.dma_start(w2t, w2[s])
                b2t = wpool.tile([C, 1], _F32, tag="b2")
                nc.sync.dma_start(b2t, b2[s][:, None])

                xT_s = xT[s].rearrange("(dc p) m -> p dc m", p=128)
                for x0, xl in _blocks(M_task, XB):
                    # per-d-chunk tiles/DMAs: 8KB-contiguous descriptors AND
                    # chunk-granular deps, so matmuls start on partial data
                    xts = []
                    for dc in range(DC):
                        xtc = xpool.tile(
                            [128, XB + 384], dt_in, tag=f"x{dc}", name=f"x_{dc}"
                        )
                        nc.sync.dma_start(
                            xtc[:, :xl], xT_s[:, dc, x0 : x0 + xl]
                        )
                        xts.append(xtc)
                    ot = opool.tile([C, XB + 384], _F32, tag="o")
                    subs = _chunks(xl, MT)
                    last_work = s == S - 1 and x0 + xl >= M_task
                    # waves of <=4 m-subtiles (PSUM bank budget); within a
                    # wave loop dc-outer so subtiles run back-to-back on the
                    # same stationary W1 chunk.
                    for w0 in range(0, len(subs), 4):
                        wave = subs[w0 : w0 + 4]
                        ps1s = [
                            psum1.tile([H, MT], _F32, tag="ps1", name=f"ps1_{j}")
                            for j in range(len(wave))
                        ]
                        last_wave = last_work and w0 + 4 >= len(subs)
                        for dc in range(DC):
                            for j, (m0, mt) in enumerate(wave):
                                nc.tensor.matmul(
                                    ps1s[j][:, :mt],
                                    w1t[:, dc, :],
                                    xts[dc][:, m0 : m0 + mt],
                                    start=(dc == 0),
                                    stop=(dc == DC - 1),
                                )
                            if not (last_wave and dc == DC - 1):
                                fill_mm(n_fill)
                        for j, (m0, mt) in enumerate(wave):
                            ht = hpool.tile([H, MT], dt_in, tag="h")
                            nc.scalar.activation(
                                ht[:, :mt], ps1s[j][:, :mt], relu, bias=b1t
                            )
                            ps2 = psum2.tile([C, MT], _F32, tag="ps2")
                            nc.tensor.matmul(
                                ps2[:, :mt], w2t, ht[:, :mt], start=True, stop=True
                            )
                            nc.vector.tensor_tensor(
                                ot[:, m0 : m0 + mt],
                                ps2[:, :mt],
                                b2t.to_broadcast([C, mt]),
                                mybir.AluOpType.add,
                            )
                    # gpsimd (SWDGE): keeps the waiting out-DMA off the SP
                    # HWDGE ring so it can't head-of-line block x-chunk DMAs
                    nc.gpsimd.dma_start(outT[s, :, x0 : x0 + xl], ot[:, :xl])
    nc.compile()
    return nc


def _prepare(x, task_id, W1, b1, W2, b2, mm_dtype=MM_DTYPE):
    """Host-side routing: returns (in_maps, idx, counts, M_task)."""
    np_in = _np_in_dt(mm_dtype)
    x = np.ascontiguousarray(np.asarray(x, dtype=np.float32))
    task_id = np.asarray(task_id).astype(np.int64)
    W1 = np.asarray(W1, dtype=np.float32)
    b1 = np.asarray(b1, dtype=np.float32)
    W2 = np.asarray(W2, dtype=np.float32)
    b2 = np.asarray(b2, dtype=np.float32)

    order = np.argsort(task_id, kind="stable")
    counts = np.bincount(task_id, minlength=T)
    starts = np.concatenate([[0], np.cumsum(counts)])
    M_task = max(128, int(-(-int(counts.max()) // 128) * 128))

    # idx[t] = sample rows for task t, padded with row 0 (discarded later)
    idx = np.zeros((T, M_task), dtype=np.int64)
    for t in range(T):
        idx[t, : counts[t]] = order[starts[t] : starts[t + 1]]

    in_maps = []
    for c in range(N_CORES):
        ts_c = [S * c + s for s in range(S)]
        rows = idx[ts_c].reshape(-1)  # [S * M_task]
        xg = x[rows].reshape(S, M_task, D)
        xT = np.ascontiguousarray(xg.transpose(0, 2, 1)).astype(np_in)
        # repack W1 [D, H] -> [128, DC*H] (partition-major for 4KB DMA rows)
        w1p = (
            W1[ts_c]
            .reshape(S, DC, 128, H)
            .transpose(0, 2, 1, 3)
            .reshape(S, 128, DC * H)
        )
        in_maps.append(
            {
                "xT": xT,
                "w1": np.ascontiguousarray(w1p).astype(np_in),
                "b1": np.ascontiguousarray(b1[ts_c]),
                "w2": np.ascontiguousarray(W2[ts_c]).astype(np_in),
                "b2": np.ascontiguousarray(b2[ts_c]),
            }
        )
    return in_maps, idx, counts, M_task


def _unshard(results, idx, counts, b_total=B):
    out = np.empty((b_total, C), dtype=np.float32)
    for c in range(N_CORES):
        yT = np.asarray(results[c]["outT"])  # [S, C, M_task]
        y = yT.transpose(0, 2, 1)  # [S, M_task, C]
        for s in range(S):
            t = S * c + s
            cnt = counts[t]
            out[idx[t, :cnt]] = y[s, :cnt]
    return out


def kernel(x, task_id, W1, b1, W2, b2):
    in_maps, idx, counts, M_task = _prepare(x, task_id, W1, b1, W2, b2)
    nc = _build(M_task)
    res = run_bass_kernel_spmd(nc, in_maps, list(range(N_CORES)))
    return _unshard(res.results, idx, counts, b_total=np.asarray(task_id).shape[0])


# revision 56
# speedup vs baseline: 1.0680x; 1.0680x over previous
"""MultiHeadClassifier (MoE routing) Trainium2 kernel.

Problem: B=65536 samples of dim D=1024, each routed by task_id to one of
T=16 two-layer heads (D->H=128 relu -> C=10). The dense reference computes
all 16 heads for every sample (275 GFLOP); here we route on the host and
compute only each sample's own head (~17 GFLOP), data-parallel with 2 tasks
per NeuronCore across 8 cores.

Strategy:
  - Host: stable-argsort samples by task; each core c owns tasks (2c, 2c+1).
    Every task segment is padded to a common M_task rows (multiple of 128) so
    the single SPMD program is identical across cores. Pad rows point at row 0
    (results discarded on unshard).
  - Host pre-transposes each core's gathered rows to xT [2, D, M_task] so the
    contraction dim D lies on SBUF partitions with contiguous DMA lines.
  - Device per (slot, m-tile of 512): 8 accumulating matmuls (W1 d-chunk
    [128,128] stationary, xT chunk [128,512] moving) -> PSUM [H=128, 512];
    ScalarE fused bias+ReLU into SBUF; one matmul with W2 [128,10] as lhsT ->
    PSUM [10,512]; ScalarE fused bias; DMA out [10, m] tiles.
  - Host scatters per-task outputs back to the original order.

MM_DTYPE selects matmul operand precision (measured on trn2, 8 cores):
  - "bf16" (default): host casts x/W to bf16; halves DMA bytes and doubles
    PE stream rate. ~70 us, rel err ~3.4e-3 (f32 accumulation in PSUM).
  - "f32r": TF32-mode matmul (fp32 bits, PE rounds mantissa internally).
    ~107 us, rel err ~2.1e-4.
  - "f32": exact fp32 two-pass matmul. ~150 us, rel err ~1.2e-7.
"""

import sys

import numpy as np

for _p in ("/opt/trn_rl_repo", "/root/.axon_site/_ro/trn_rl_repo"):
    if _p not in sys.path:
        sys.path.append(_p)

import concourse.bacc as bacc
import concourse.mybir as mybir
from concourse.bass_utils import run_bass_kernel_spmd
from concourse.tile import TileContext

B, D, T, H, C = 65536, 1024, 16, 128, 10
N_CORES = 8
S = T // N_CORES  # task slots per core = 2
DC = D // 128  # d-chunks of 128 = 8
MT = 512  # m-tile (max fp32 moving free dim)
X_BUFS = 2

MM_DTYPE = "bf16"

_F32 = mybir.dt.float32


def _mm_dt(mm_dtype):
    return {
        "f32": mybir.dt.float32,
        "f32r": mybir.dt.float32r,
        "bf16": mybir.dt.bfloat16,
    }[mm_dtype]


def _np_in_dt(mm_dtype):
    import ml_dtypes

    return np.dtype(ml_dtypes.bfloat16) if mm_dtype == "bf16" else np.dtype(np.float32)


def _chunks(total, step, merge_tail=0):
    """Split [0, total) into (start, len) chunks of `step` plus remainder.

    A final chunk smaller than merge_tail is merged into the previous one.
    """
    out = []
    p = 0
    while p < total:
        c = min(step, total - p)
        out.append((p, c))
        p += c
    if merge_tail and len(out) > 1 and out[-1][1] < merge_tail:
        p0, c0 = out[-2]
        p1, c1 = out[-1]
        out[-2:] = [(p0, c0 + c1)]
    return out


def _blocks(total, step):
    """Like _chunks but ends with a small (<=512) final block so the
    compute tail after the last DMA is short."""
    out = []
    p = 0
    rem = total
    while rem > 0:
        if rem <= 512 or rem <= step:
            c = rem
        elif rem <= step + 512:
            c = rem - 512
        else:
            c = step
        out.append((p, c))
        p += c
        rem -= c
    return out


def _build(M_task, mm_dtype=MM_DTYPE):
    dt_in = _mm_dt(mm_dtype)
    # x DMA block (samples)
    XB = 2048
    x_bufs = 3 if mm_dtype == "bf16" else X_BUFS
    # bf16 matmuls double throughput when the HAM clock-gate is open; keep
    # the PE duty high with warmup + filler matmuls. f32r matmuls are
    # SBUF-stream-bound (same speed warm or cold) -> fillers only hurt.
    n_warmup = 16 if mm_dtype == "bf16" else 0
    n_fill = 0
    nc = bacc.Bacc(None, target_bir_lowering=False)
    xT = nc.declare_dram_parameter("xT", [S, D, M_task], dt_in, isOutput=False)
    # w1 arrives host-repacked as [S, 128, DC*H]: partition-major, 4KB/row
    w1 = nc.declare_dram_parameter("w1", [S, 128, DC * H], dt_in, isOutput=False)
    b1 = nc.declare_dram_parameter("b1", [S, H], _F32, isOutput=False)
    w2 = nc.declare_dram_parameter("w2", [S, H, C], dt_in, isOutput=False)
    b2 = nc.declare_dram_parameter("b2", [S, C], _F32, isOutput=False)
    outT = nc.declare_dram_parameter("outT", [S, C, M_task], _F32, isOutput=True)

    relu = mybir.ActivationFunctionType.Relu

    with TileContext(nc) as tc:
        with (
            tc.tile_pool(name="wpool", bufs=2) as wpool,
            tc.tile_pool(name="xpool", bufs=x_bufs) as xpool,
            tc.tile_pool(name="hpool", bufs=6) as hpool,
            tc.tile_pool(name="opool", bufs=2) as opool,
            tc.tile_pool(name="warm", bufs=1) as warm,
            tc.tile_pool(name="psum1", bufs=5, space="PSUM") as psum1,
            tc.tile_pool(name="psum2", bufs=2, space="PSUM") as psum2,
            tc.tile_pool(name="psumw", bufs=1, space="PSUM") as psumw,
        ):  # PSUM banks: 5 + 2 + 1 = 8
            # PE warmup: dummy matmuls release the HAM clock-gate (~3.4us of
            # sustained PE busy) while the first x block streams in.
            if n_warmup or n_fill:
                wsrc = warm.tile([128, MT], _F32, tag="wsrc")
                nc.gpsimd.memset(wsrc[:], 0.0)
                wv = wsrc[:].bitcast(dt_in)
                wps = psumw.tile([128, MT], _F32, tag="wps")

            def fill_mm(n):
                for _ in range(n):
                    nc.tensor.matmul(
                        wps[:], wv[:, :128], wv[:, :MT], start=True, stop=True
                    )

            fill_mm(n_warmup)
            for s in range(S):
                # scalar HWDGE ring: weight loads never queue behind (or
                # ahead of) the x-chunk stream on the sync ring
                w1t = wpool.tile([128, DC, H], dt_in, tag="w1")
                nc.scalar.dma_start(
                    w1t, w1[s].rearrange("p (dc h) -> p dc h", dc=DC)
                )
                b1t = wpool.tile([H, 1], _F32, tag="b1")
                nc.scalar.dma_start(b1t, b1[s][:, None])
                w2t = wpool.tile([H, C], dt_in, tag="w2")
                nc.scalar.dma_start(w2t, w2[s])
                b2t = wpool.tile([C, 1], _F32, tag="b2")
                nc.scalar.dma_start(b2t, b2[s][:, None])

                xT_s = xT[s].rearrange("(dc p) m -> p dc m", p=128)
                for x0, xl in _blocks(M_task, XB):
                    # per-d-chunk tiles/DMAs: 8KB-contiguous descriptors AND
                    # chunk-granular deps, so matmuls start on partial data
                    xts = []
                    for dc in range(DC):
                        xtc = xpool.tile(
                            [128, XB + 384], dt_in, tag=f"x{dc}", name=f"x_{dc}"
                        )
                        nc.sync.dma_start(
                            xtc[:, :xl], xT_s[:, dc, x0 : x0 + xl]
                        )
                        xts.append(xtc)
                    ot = opool.tile([C, XB + 384], _F32, tag="o")
                    subs = _chunks(xl, MT)
                    last_work = s == S - 1 and x0 + xl >= M_task
                    # waves of <=4 m-subtiles (PSUM bank budget); within a
                    # wave loop dc-outer so subtiles run back-to-back on the
                    # same stationary W1 chunk.
                    for w0 in range(0, len(subs), 4):
                        wave = subs[w0 : w0 + 4]
                        ps1s = [
                            psum1.tile([H, MT], _F32, tag="ps1", name=f"ps1_{j}")
                            for j in range(len(wave))
                        ]
                        last_wave = last_work and w0 + 4 >= len(subs)
                        for dc in range(DC):
                            for j, (m0, mt) in enumerate(wave):
                                nc.tensor.matmul(
                                    ps1s[j][:, :mt],
                                    w1t[:, dc, :],
                                    xts[dc][:, m0 : m0 + mt],
                                    start=(dc == 0),
                                    stop=(dc == DC - 1),
                                )
                            if not (last_wave and dc == DC - 1):
                                fill_mm(n_fill)
                        for j, (m0, mt) in enumerate(wave):
                            ht = hpool.tile([H, MT], dt_in, tag="h")
                            nc.scalar.activation(
                                ht[:, :mt], ps1s[j][:, :mt], relu, bias=b1t
                            )
                            ps2 = psum2.tile([C, MT], _F32, tag="ps2")
                            nc.tensor.matmul(
                                ps2[:, :mt], w2t, ht[:, :mt], start=True, stop=True
                            )
                            nc.vector.tensor_tensor(
                                ot[:, m0 : m0 + mt],
                                ps2[:, :mt],
                                b2t.to_broadcast([C, mt]),
                                mybir.AluOpType.add,
                            )
                    # gpsimd (SWDGE): keeps the waiting out-DMA off the SP
                    # HWDGE ring so it can't head-of-line block x-chunk DMAs
                    nc.gpsimd.dma_start(outT[s, :, x0 : x0 + xl], ot[:, :xl])
    nc.compile()
    return nc


def _prepare(x, task_id, W1, b1, W2, b2, mm_dtype=MM_DTYPE):
    """Host-side routing: returns (in_maps, idx, counts, M_task)."""
    np_in = _np_in_dt(mm_dtype)
    x = np.ascontiguousarray(np.asarray(x, dtype=np.float32))
    task_id = np.asarray(task_id).astype(np.int64)
    W1 = np.asarray(W1, dtype=np.float32)
    b1 = np.asarray(b1, dtype=np.float32)
    W2 = np.asarray(W2, dtype=np.float32)
    b2 = np.asarray(b2, dtype=np.float32)

    order = np.argsort(task_id, kind="stable")
    counts = np.bincount(task_id, minlength=T)
    starts = np.concatenate([[0], np.cumsum(counts)])
    M_task = max(128, int(-(-int(counts.max()) // 128) * 128))

    # idx[t] = sample rows for task t, padded with row 0 (discarded later)
    idx = np.zeros((T, M_task), dtype=np.int64)
    for t in range(T):
        idx[t, : counts[t]] = order[starts[t] : starts[t + 1]]

    in_maps = []
    for c in range(N_CORES):
        ts_c = [S * c + s for s in range(S)]
        rows = idx[ts_c].reshape(-1)  # [S * M_task]
        xg = x[rows].reshape(S, M_task, D)
        xT = np.ascontiguousarray(xg.transpose(0, 2, 1)).astype(np_in)
        # repack W1 [D, H] -> [128, DC*H] (partition-major for 4KB DMA rows)
        w1p = (
            W1[ts_c]
            .reshape(S, DC, 128, H)
            .transpose(0, 2, 1, 3)
            .reshape(S, 128, DC * H)
        )
        in_maps.append(
            {
                "xT": xT,
                "w1": np.ascontiguousarray(w1p).astype(np_in),
                "b1": np.ascontiguousarray(b1[ts_c]),
                "w2": np.ascontiguousarray(W2[ts_c]).astype(np_in),
                "b2": np.ascontiguousarray(b2[ts_c]),
            }
        )
    return in_maps, idx, counts, M_task


def _unshard(results, idx, counts, b_total=B):
    out = np.empty((b_total, C), dtype=np.float32)
    for c in range(N_CORES):
        yT = np.asarray(results[c]["outT"])  # [S, C, M_task]
        y = yT.transpose(0, 2, 1)  # [S, M_task, C]
        for s in range(S):
            t = S * c + s
            cnt = counts[t]
            out[idx[t, :cnt]] = y[s, :cnt]
    return out


def kernel(x, task_id, W1, b1, W2, b2):
    in_maps, idx, counts, M_task = _prepare(x, task_id, W1, b1, W2, b2)
    nc = _build(M_task)
    try:
        res = run_bass_kernel_spmd(nc, in_maps, list(range(N_CORES)))
    except Exception:
        # transient NRT device hiccups (e.g. NRT_EXEC_UNIT_UNRECOVERABLE)
        # have been observed to succeed on retry
        res = run_bass_kernel_spmd(nc, in_maps, list(range(N_CORES)))
    return _unshard(res.results, idx, counts, b_total=np.asarray(task_id).shape[0])
